# revision 1
# baseline (speedup 1.0000x reference)
"""Trainium2 Bass kernel for a dense transformer block (pre-LN attention + GELU MLP).

Strategy: data-parallel over batch across 8 NeuronCores (2 batches/core, no
collectives).  Per core: token-major residual stream with feature-major
activations for matmuls (PE-transpose at the two LayerNorms), fp32r matmuls
(full PE rate), softmax without max-subtraction (scores are O(1) bounded by
construction), PV matmul with a ones-column on V to produce row-sums for free.
"""

import numpy as np

import concourse.bass as bass
import concourse.mybir as mybir
import concourse.tile as tile
from concourse import bacc, bass_utils
from concourse.masks import make_identity

# Problem shape (hardcoded per spec nn_Block_58652073394865)
B, S, D, H, F = 16, 577, 1024, 16, 4096
DH = D // H
NCORES = 8
BL = B // NCORES        # batches per core
P = 128
KK = D // P             # 8 chunks of the model dim
FK = F // P             # 32 chunks of the mlp dim
EPS = 1e-6

# fp32r matmuls require even free-dim counts, so pad tokens 577 -> 578 (one
# zeroed pad token) and use even, overlapping moving-token chunks.
SP = 578
TT = [(0, 128), (128, 128), (256, 128), (384, 128), (512, 66)]   # token tiles (incl pad)
QC = [(0, 290), (288, 290)]                                      # moving-token chunks (even, >=256)
DC = [(0, 512), (512, 512)]                                      # model-dim 512 chunks
VS = 66                                                          # per-head stride in v (64 v + 1 ones + 1 pad)

F32 = mybir.dt.float32
F32R = mybir.dt.float32r
AF = mybir.ActivationFunctionType
OP = mybir.AluOpType

WEIGHT_NAMES = [
    "ln1_g", "ln1_b", "wq", "bq", "wk", "bk", "wv", "bv", "wo", "bo",
    "ln2_g", "ln2_b", "w1", "b1", "w2", "b2",
]

_NC_CACHE = None
# CoreSim doesn't implement the Gelu LUT; tests may swap this for AF.Tanh
_GELU = AF.Gelu


def _build():
    nc = bacc.Bacc("TRN2", target_bir_lowering=False, debug=False,
                   num_devices=NCORES)

    x_d = nc.dram_tensor("x", [BL, S, D], F32, kind="ExternalInput").ap()
    y_d = nc.dram_tensor("y", [BL, S, D], F32, kind="ExternalOutput").ap()
    # weights consumed by matmuls -> declare fp32r (same bits as fp32)
    wq_d = nc.dram_tensor("wq", [D, D], F32R, kind="ExternalInput").ap()
    wk_d = nc.dram_tensor("wk", [D, D], F32R, kind="ExternalInput").ap()
    wv_d = nc.dram_tensor("wv", [D, D], F32R, kind="ExternalInput").ap()
    wo_d = nc.dram_tensor("wo", [D, D], F32R, kind="ExternalInput").ap()
    w1_d = nc.dram_tensor("w1", [D, F], F32R, kind="ExternalInput").ap()
    w2_d = nc.dram_tensor("w2", [F, D], F32R, kind="ExternalInput").ap()
    bv_d = nc.dram_tensor("bv", [D], F32R, kind="ExternalInput").ap()   # folded via K=1 matmul
    bo_d = nc.dram_tensor("bo", [D], F32R, kind="ExternalInput").ap()   # folded via K=1 matmul
    bq_d = nc.dram_tensor("bq", [D], F32, kind="ExternalInput").ap()
    bk_d = nc.dram_tensor("bk", [D], F32, kind="ExternalInput").ap()
    b1_d = nc.dram_tensor("b1", [F], F32, kind="ExternalInput").ap()
    b2_d = nc.dram_tensor("b2", [D], F32, kind="ExternalInput").ap()
    g1_d = nc.dram_tensor("ln1_g", [D], F32, kind="ExternalInput").ap()
    gb1_d = nc.dram_tensor("ln1_b", [D], F32, kind="ExternalInput").ap()
    g2_d = nc.dram_tensor("ln2_g", [D], F32, kind="ExternalInput").ap()
    gb2_d = nc.dram_tensor("ln2_b", [D], F32, kind="ExternalInput").ap()

    wq_r = wq_d.rearrange("(ko p) d -> p ko d", p=P)
    wk_r = wk_d.rearrange("(ko p) d -> p ko d", p=P)
    wv_r = wv_d.rearrange("(ko p) d -> p ko d", p=P)
    wo_r = wo_d.rearrange("(ko p) d -> p ko d", p=P)
    w1_r = w1_d.rearrange("(ko p) d -> p ko d", p=P)
    w2_r = w2_d.rearrange("(ko p) d -> p ko d", p=P)

    with tile.TileContext(nc) as tc:
        with tc.tile_pool(name="const", bufs=1) as cpool, \
             tc.tile_pool(name="resid", bufs=2) as rpool, \
             tc.tile_pool(name="fmbuf", bufs=1) as fmpool, \
             tc.tile_pool(name="ostg", bufs=4) as opool, \
             tc.tile_pool(name="lnp", bufs=2) as lnpool, \
             tc.tile_pool(name="psA", bufs=4, space="PSUM") as psA:

            # ---- constants / small params ----
            # tiles pad to 4KB/partition: pack the small params into few tiles
            cA = cpool.tile([P, 7 * KK + FK], F32, tag="cA")
            bq_sb = cA[:, 0:KK]
            bk_sb = cA[:, KK:2 * KK]
            b2_sb = cA[:, 2 * KK:3 * KK]
            g1_sb = cA[:, 3 * KK:4 * KK]
            gb1_sb = cA[:, 4 * KK:5 * KK]
            g2_sb = cA[:, 5 * KK:6 * KK]
            gb2_sb = cA[:, 6 * KK:7 * KK]
            b1_sb = cA[:, 7 * KK:7 * KK + FK]
            nc.sync.dma_start(bq_sb, bq_d.rearrange("(m p) -> p m", p=P))
            nc.sync.dma_start(bk_sb, bk_d.rearrange("(m p) -> p m", p=P))
            nc.sync.dma_start(b2_sb, b2_d.rearrange("(m p) -> p m", p=P))
            nc.sync.dma_start(g1_sb, g1_d.rearrange("(c p) -> p c", p=P))
            nc.sync.dma_start(gb1_sb, gb1_d.rearrange("(c p) -> p c", p=P))
            nc.sync.dma_start(g2_sb, g2_d.rearrange("(c p) -> p c", p=P))
            nc.sync.dma_start(gb2_sb, gb2_d.rearrange("(c p) -> p c", p=P))
            nc.sync.dma_start(b1_sb, b1_d.rearrange("(m p) -> p m", p=P))

            cB = cpool.tile([P, P + 2], F32, tag="cB")
            ident = cB[:, 0:P]
            epsap = cB[:, P:P + 1]
            onec_f = cB[:, P + 1:P + 2]
            make_identity(nc, ident)
            nc.vector.memset(epsap, EPS)
            nc.vector.memset(onec_f, 1.0)

            ident_r = cpool.tile([P, P], F32R, tag="ident_r")
            nc.vector.tensor_copy(ident_r[:], ident)

            ones_f = cpool.tile([1, P], F32, tag="ones_f")
            nc.vector.memset(ones_f[:], 1.0)
            cD = cpool.tile([1, P + 2 * D], F32R, tag="cD")
            ones_r = cD[:, 0:P]
            t_bo = cD[:, P:P + D]
            t_bv = cD[:, P + D:P + 2 * D]
            nc.vector.tensor_copy(ones_r, ones_f[:])
            nc.sync.dma_start(t_bo, bo_d[None, :])
            nc.sync.dma_start(t_bv, bv_d[None, :])

            # token-major layernorm -> feature-major normalized output
            def ln_new_stats(ln_pool):
                stats = ln_pool.tile([P, 20], F32, tag="stats")
                # last token tile covers only 66 partitions; keep the rest defined
                nc.vector.memset(stats[:, 0:5], 0.0)
                nc.vector.memset(stats[:, 5:10], 1.0)
                return stats

            def ln_tile_stats(ln_pool, stats, src, ti, pt):
                negmu = stats[:, 0:5]
                varD = stats[:, 5:10]
                nc.vector.tensor_reduce(
                    negmu[:pt, ti:ti + 1], src[:pt, ti],
                    mybir.AxisListType.X, OP.add)
                nc.vector.tensor_scalar_mul(
                    negmu[:pt, ti:ti + 1], negmu[:pt, ti:ti + 1], -1.0 / D)
                scr = ln_pool.tile([P, D], F32R, tag="xn_tm", bufs=3)
                nc.scalar.activation(
                    scr[:pt], src[:pt, ti], AF.Square,
                    bias=negmu[:pt, ti:ti + 1], accum_out=varD[:pt, ti:ti + 1])

            def ln_finalize(stats, lo, hi):
                # rsig for tile range [lo, hi)
                nc.scalar.activation(stats[:, 10 + lo:10 + hi],
                                     stats[:, 5 + lo:5 + hi], AF.Sqrt,
                                     scale=1.0 / D, bias=epsap[:])
                nc.vector.reciprocal(stats[:, 15 + lo:15 + hi],
                                     stats[:, 10 + lo:10 + hi])

            def ln_apply_tiles(ln_pool, stats, src, g_sb, gb_sb, dst_fm, tis):
                negmu = stats[:, 0:5]
                rsig = stats[:, 15:20]
                for ti in tis:
                    t0, pt = TT[ti]
                    xn = ln_pool.tile([P, D], F32R, tag="xn_tm", bufs=3)
                    nc.vector.tensor_scalar(
                        xn[:pt], src[:pt, ti],
                        negmu[:pt, ti:ti + 1], rsig[:pt, ti:ti + 1],
                        OP.add, OP.mult)
                    for kk in range(KK):
                        pst = psA.tile([P, 512], F32R, tag="pA")
                        nc.tensor.transpose(
                            pst[:, :pt], xn[:pt, kk * P:(kk + 1) * P],
                            ident_r[:pt, :pt])
                        nc.vector.scalar_tensor_tensor(
                            dst_fm[:, kk, t0:t0 + pt], pst[:, :pt],
                            g_sb[:, kk:kk + 1],
                            gb_sb[:, kk:kk + 1].to_broadcast((P, pt)),
                            OP.mult, OP.add)

            def layer_norm_fm(ln_pool, src, g_sb, gb_sb, dst_fm):
                stats = ln_new_stats(ln_pool)
                for ti, (t0, pt) in enumerate(TT):
                    ln_tile_stats(ln_pool, stats, src, ti, pt)
                # finalize tile 0 alone so its transposes start after one x-tile
                ln_finalize(stats, 0, 1)
                ln_apply_tiles(ln_pool, stats, src, g_sb, gb_sb, dst_fm, (0,))
                ln_finalize(stats, 1, 4)
                ln_apply_tiles(ln_pool, stats, src, g_sb, gb_sb, dst_fm, (1, 2, 3))
                ln_finalize(stats, 4, 5)
                ln_apply_tiles(ln_pool, stats, src, g_sb, gb_sb, dst_fm, (4,))

            for b in range(BL):
                xn_fm = fmpool.tile([P, KK, SP], F32R, tag="xn_fm")
                xb = rpool.tile([P, 5, D], F32, tag="resid")

                # ---- stage A: load x (token-major); zero the pad token row ----
                # (engine start-partition must be a multiple of 32: zero 64..127
                # first, then the DMA rewrites the real rows 0..64)
                nc.vector.memset(xb[64:, 4, :], 0.0)
                for ti, (t0, pt) in enumerate(TT):
                    rp = min(pt, S - t0)   # real (non-pad) tokens in this tile
                    nc.sync.dma_start(xb[:rp, ti], x_d[b, t0:t0 + rp, :])

                # ---- stage B: LN1 -> xn_fm ----
                layer_norm_fm(lnpool, xb, g1_sb, gb1_sb, xn_fm)

                with tc.tile_pool(name="attn", bufs=1) as apool, \
                     tc.tile_pool(name="wblk", bufs=2) as wpool:
                    q_fm = apool.tile([P, KK, SP], F32R, tag="q")
                    k_fm = apool.tile([P, KK, SP], F32R, tag="k")
                    v_sb = apool.tile([P, 5, H * VS], F32R, tag="v")
                    ctx_fm = apool.tile([P, KK, SP], F32R, tag="ctx")

                    # col 64 of each head's stride-66 group = 1 (rowsum trick),
                    # col 65 = 0 (fp32r even-M pad).  The pad token's whole v
                    # row (tile 4, partition 65) must be zero: zero partitions
                    # 64.. first, later writes refill only the real rows.
                    v_hc = v_sb[:].rearrange("p t (h c) -> p t h c", c=VS)
                    # memset can't target fp32r; zero via a uint32 view
                    nc.vector.memset(v_hc[64:, 4:5].bitcast(mybir.dt.uint32), 0)
                    nc.vector.memset(v_hc[:, :, :, 65:66].bitcast(mybir.dt.uint32), 0)
                    nc.vector.tensor_copy(
                        v_hc[:, 0:4, :, 64:65],
                        onec_f[:, :, None, None].to_broadcast((P, 4, H, 1)))
                    nc.vector.tensor_copy(
                        v_hc[:65, 4:5, :, 64:65],
                        onec_f[:65, :, None, None].to_broadcast((65, 1, H, 1)))

                    # ---- stage C/D interleaved: projections + attention ----
                    # blk covers q/k m-tiles 4*blk..4*blk+3 and v heads
                    # 8*blk..8*blk+7 == attention heads 8*blk..8*blk+7, so each
                    # half's projections feed its attention while the NEXT
                    # half's projection matmuls fill the exp-bound PE idle.
                    def emit_qk(blk):
                        for w_r, bias_sb, dst in ((wq_r, bq_sb, q_fm), (wk_r, bk_sb, k_fm)):
                            wb = wpool.tile([P, KK, 512], F32R, tag="wblk")
                            nc.sync.dma_start(wb[:], w_r[:, :, blk * 512:(blk + 1) * 512])
                            for mi in range(4):
                                m = blk * 4 + mi
                                for (q0, qn) in QC:
                                    ps = psA.tile([P, 512], F32, tag="pA")
                                    for kk in range(KK):
                                        nc.tensor.matmul(
                                            ps[:, :qn],
                                            wb[:, kk, mi * P:(mi + 1) * P],
                                            xn_fm[:, kk, q0:q0 + qn],
                                            start=(kk == 0), stop=(kk == KK - 1))
                                    nc.scalar.activation(
                                        dst[:, m, q0:q0 + qn], ps[:, :qn],
                                        AF.Identity, bias=bias_sb[:, m:m + 1])

                    def emit_v(ci):
                        c0, cn = DC[ci]
                        wb = wpool.tile([P, KK, 512], F32R, tag="wblk")
                        nc.sync.dma_start(wb[:], wv_r[:, :, c0:c0 + cn])
                        for ti, (t0, pt) in enumerate(TT):
                            ps = psA.tile([P, 512], F32, tag="pA")
                            for kk in range(KK):
                                nc.tensor.matmul(
                                    ps[:pt], xn_fm[:, kk, t0:t0 + pt],
                                    wb[:, kk, :], start=(kk == 0), stop=False)
                            nc.tensor.matmul(
                                ps[:pt], ones_r[:, :pt], t_bv[:, c0:c0 + cn],
                                start=False, stop=True)
                            rp = min(pt, S - t0)
                            nc.vector.tensor_copy(
                                v_sb[:rp, ti].rearrange("p (h c) -> p h c", c=VS)[:, ci * 8:(ci + 1) * 8, 0:64],
                                ps[:rp, :cn].rearrange("p (h c) -> p h c", c=64))

                    def emit_attn(h):
                        hrow = (h % 2) * 64
                        kkh = h // 2
                        for qi, (q0, qn) in enumerate(QC):
                            es = apool.tile([P, 5, qn], F32R, tag=f"es{qi}")
                            # pair the 5 score tiles into 2-bank psum groups so
                            # each Exp covers 2 tiles (halves the per-op cost)
                            for pair in ((0, 1), (2, 3), (4,)):
                                pg = psA.tile([P, 2, 512], F32, tag="pS", bufs=2)
                                for j, kt in enumerate(pair):
                                    t0, ptk = TT[kt]
                                    nc.tensor.matmul(
                                        pg[:ptk, j, :qn],
                                        k_fm[hrow:hrow + 64, kkh, t0:t0 + ptk],
                                        q_fm[hrow:hrow + 64, kkh, q0:q0 + qn],
                                        start=True, stop=True)
                                npair = len(pair)
                                prow = TT[pair[0]][1]   # 128 for full pairs, 66 for (4,)
                                nc.scalar.activation(
                                    es[:prow, pair[0]:pair[0] + npair, :],
                                    pg[:prow, :npair, :qn],
                                    AF.Exp, scale=1.0 / np.sqrt(DH))
                            pc = psA.tile([VS, 512], F32, tag="pA")
                            for kt, (t0, ptk) in enumerate(TT):
                                nc.tensor.matmul(
                                    pc[:, :qn],
                                    v_sb[:ptk, kt, h * VS:(h + 1) * VS],
                                    es[:ptk, kt, :],
                                    start=(kt == 0), stop=(kt == 4))
                            rc = apool.tile([1, 290], F32, tag="rc", bufs=2)
                            nc.vector.reciprocal(rc[:, :qn], pc[64:65, :qn])
                            rb = apool.tile([64, 290], F32, tag="rb", bufs=2)
                            nc.gpsimd.partition_broadcast(rb[:, :qn], rc[:, :qn])
                            nc.vector.tensor_tensor(
                                ctx_fm[hrow:hrow + 64, kkh, q0:q0 + qn],
                                pc[0:64, :qn], rb[:, :qn], OP.mult)

                    emit_qk(0)
                    emit_v(0)
                    for h in range(8):
                        emit_attn(h)
                    emit_qk(1)
                    emit_v(1)
                    for h in range(8, H):
                        emit_attn(h)

                    # ---- stage E: output projection + residual -> x2,
                    # with LN2 folded in per-tile ----
                    x2 = rpool.tile([P, 5, D], F32, tag="resid")
                    xn2_fm = fmpool.tile([P, KK, SP], F32R, tag="xn_fm")
                    stats2 = ln_new_stats(lnpool)
                    for ci, (c0, cn) in enumerate(DC):
                        wb = wpool.tile([P, KK, 512], F32R, tag="wblk")
                        nc.sync.dma_start(wb[:], wo_r[:, :, c0:c0 + cn])
                        for ti, (t0, pt) in enumerate(TT):
                            ps = psA.tile([P, 512], F32, tag="pA")
                            for kk in range(KK):
                                nc.tensor.matmul(
                                    ps[:pt], ctx_fm[:, kk, t0:t0 + pt],
                                    wb[:, kk, :], start=(kk == 0), stop=False)
                            nc.tensor.matmul(
                                ps[:pt], ones_r[:, :pt], t_bo[:, c0:c0 + cn],
                                start=False, stop=True)
                            nc.vector.scalar_tensor_tensor(
                                x2[:pt, ti, c0:c0 + cn], ps[:pt], 0.0,
                                xb[:pt, ti, c0:c0 + cn], OP.add, OP.add)
                            if ci == len(DC) - 1:
                                # x2 tile complete: fold its LN2 stats in now
                                ln_tile_stats(lnpool, stats2, x2, ti, pt)



                # ---- stage F: LN2 apply ----
                ln_finalize(stats2, 0, 4)
                ln_apply_tiles(lnpool, stats2, x2, g2_sb, gb2_sb, xn2_fm, (0, 1, 2, 3))
                ln_finalize(stats2, 4, 5)
                ln_apply_tiles(lnpool, stats2, x2, g2_sb, gb2_sb, xn2_fm, (4,))

                # ---- stage G: MLP ----
                with tc.tile_pool(name="mlp", bufs=1) as mpool, \
                     tc.tile_pool(name="wmlp", bufs=2) as mwpool:
                    h1 = mpool.tile([P, FK, SP], F32R, tag="h1")
                    _psc = [0]

                    def mlp_psum():
                        # pS's 2x2 banks are idle during MLP: every 3rd group
                        # borrows one -> 6 accumulation groups in flight
                        _psc[0] += 1
                        if _psc[0] % 3 == 0:
                            t = psA.tile([P, 2, 512], F32, tag="pS", bufs=2,
                                         name="ps_alt")
                            return t[:, 0]
                        return psA.tile([P, 512], F32, tag="pA", name="ps_a")

                    for blk in range(8):
                        wb = mwpool.tile([P, KK, 512], F32R, tag="wmlp")
                        nc.sync.dma_start(wb[:], w1_r[:, :, blk * 512:(blk + 1) * 512])
                        for mi in range(4):
                            m = blk * 4 + mi
                            for (q0, qn) in QC:
                                ps = mlp_psum()
                                for kk in range(KK):
                                    nc.tensor.matmul(
                                        ps[:, :qn],
                                        wb[:, kk, mi * P:(mi + 1) * P],
                                        xn2_fm[:, kk, q0:q0 + qn],
                                        start=(kk == 0), stop=(kk == KK - 1))
                                nc.scalar.activation(
                                    h1[:, m, q0:q0 + qn], ps[:, :qn],
                                    _GELU, bias=b1_sb[:, m:m + 1])
                    mlp_fm = mpool.tile([P, KK, SP], F32R, tag="mlp_fm")
                    for m in range(KK):
                        wb = mwpool.tile([P, FK, P], F32R, tag="wmlp")
                        nc.sync.dma_start(wb[:], w2_r[:, :, m * P:(m + 1) * P])
                        for (q0, qn) in QC:
                            ps = mlp_psum()
                            for kk2 in range(FK):
                                nc.tensor.matmul(
                                    ps[:, :qn], wb[:, kk2],
                                    h1[:, kk2, q0:q0 + qn],
                                    start=(kk2 == 0), stop=(kk2 == FK - 1))
                            nc.vector.tensor_scalar_add(
                                mlp_fm[:, m, q0:q0 + qn], ps[:, :qn],
                                b2_sb[:, m:m + 1])
                        # this m's feature rows are complete: transpose back to
                        # token-major, add residual, store (interleaves with the
                        # next m's w2 matmuls)
                        for ti, (t0, pt) in enumerate(TT):
                            rp = min(pt, S - t0)   # skip the pad token on store
                            ps = psA.tile([P, 512], F32R, tag="pA")
                            nc.tensor.transpose(
                                ps[:pt, :P], mlp_fm[:, m, t0:t0 + pt], ident_r[:])
                            og = opool.tile([P, P], F32, tag="ostg", bufs=6)
                            nc.vector.scalar_tensor_tensor(
                                og[:pt], ps[:pt, :P], 0.0,
                                x2[:pt, ti, m * P:(m + 1) * P], OP.add, OP.add)
                            nc.sync.dma_start(
                                y_d[b, t0:t0 + rp, m * P:(m + 1) * P], og[:rp])

    nc.compile()
    return nc


def _get_nc():
    global _NC_CACHE
    if _NC_CACHE is None:
        _NC_CACHE = _build()
    return _NC_CACHE


def kernel(**inputs):
    nc = _get_nc()
    x = np.ascontiguousarray(np.asarray(inputs["x"], dtype=np.float32))
    shared = {
        n: np.ascontiguousarray(np.asarray(inputs[n], dtype=np.float32))
        for n in WEIGHT_NAMES
    }
    in_maps = []
    for i in range(NCORES):
        m = dict(shared)
        m["x"] = np.ascontiguousarray(x[i * BL:(i + 1) * BL])
        in_maps.append(m)
    res = bass_utils.run_bass_kernel_spmd(nc, in_maps, core_ids=list(range(NCORES)))
    y = np.concatenate([res.results[i]["y"] for i in range(NCORES)], axis=0)
    return y.astype(np.float32)



# revision 6
# speedup vs baseline: 1.1594x; 1.1594x over previous
"""Trainium2 Bass kernel for a dense transformer block (pre-LN attention + GELU MLP).

Strategy: data-parallel over batch across 8 NeuronCores (2 batches/core).
Per core, the two batches are software-pipelined so the Act-bound softmax
phase of one batch overlaps the PE-bound MLP/projection phases of the other.

Precision plan (validated empirically, final rel-err ~9e-3 vs 2e-2 gate):
  - residual stream fp32/bf16, LN stats fp32
  - LN gains/biases folded into the weights host-side (device LN is pure
    (x-mu)*rsig); rsqrt computed via Ln+Exp so every Act function used
    around attention lives in one LUT set (no table thrash)
  - QKV/O projections and the MLP w1 matmul: fp8e4m3 with DoubleRow
    (2 K-tiles per instruction), weights pre-scaled x32 (x32*16 for wo)
    host-side to escape fp8 subnormals, descaled for free downstream
  - w1 uses a 3-term hi/lo compensated fp8 product (err ~bf16)
  - attention scores and PV in fp8 (probs = exp(s)/16 to stay in fp8 range;
    normalization by the quantized-prob rowsum via a 1/16-ones column in V)
  - w2 matmul in bf16 (moving operand = weights; token-major output kills
    the output transpose and makes the residual add direct)
"""

import numpy as np
import ml_dtypes

import concourse.bass as bass
import concourse.mybir as mybir
import concourse.tile as tile
from concourse import bacc, bass_utils
from concourse.masks import make_identity

# Problem shape (hardcoded per spec nn_Block_58652073394865)
B, S, D, H, F = 16, 577, 1024, 16, 4096
DH = D // H
NCORES = 8
BL = B // NCORES
P = 128
KK = D // P              # 8
FK = F // P              # 32
EPS = 1e-6

SP = 578                 # padded tokens (577 + 1 zero pad)
SPAD = 592               # row stride for DR-operand feature-major tensors (%16==0)
ESP = 304                # es row stride (%16==0)
TT = [(0, 128), (128, 128), (256, 128), (384, 128), (512, 66)]
QC = [(0, 290), (290, 288)]
DC = [(0, 512), (512, 512)]
DC4 = [(0, 256), (256, 256), (512, 256), (768, 256)]
PW2 = 64                 # w2 output chunk width
VS = 66                  # per-head stride in v (64 v + 1 ones + 1 spare)
WS = 32.0                # fp8 weight pre-scale
CTXS = 16.0              # ctx pre-scale (via 1/16 ones column)
EXPB = -2.772588722239781  # -ln(16): probs = exp(s)*1/16

F32 = mybir.dt.float32
BF16 = mybir.dt.bfloat16
FP8 = mybir.dt.float8e4
AF = mybir.ActivationFunctionType
OP = mybir.AluOpType
DR = mybir.MatmulPerfMode.DoubleRow

_NC_CACHE = None
# CoreSim doesn't implement the Gelu LUT; tests may swap this for AF.Tanh
_GELU = AF.Gelu


def _build():
    nc = bacc.Bacc("TRN2", target_bir_lowering=False, debug=False,
                   num_devices=NCORES)

    x_d = nc.dram_tensor("x", [BL, S, D], F32, kind="ExternalInput").ap()
    y_d = nc.dram_tensor("y", [BL, S, D], F32, kind="ExternalOutput").ap()
    wq_d = nc.dram_tensor("wq8", [P, KK, D], FP8, kind="ExternalInput").ap()
    wk_d = nc.dram_tensor("wk8", [P, KK, D], FP8, kind="ExternalInput").ap()
    wv_d = nc.dram_tensor("wv8", [P, KK, D], FP8, kind="ExternalInput").ap()
    wo_d = nc.dram_tensor("wo8", [P, KK, D], FP8, kind="ExternalInput").ap()
    w1h_d = nc.dram_tensor("w1h", [P, KK, F], FP8, kind="ExternalInput").ap()
    w1l_d = nc.dram_tensor("w1l", [P, KK, F], FP8, kind="ExternalInput").ap()
    w2_d = nc.dram_tensor("w2b", [P, FK, D], BF16, kind="ExternalInput").ap()
    bq_d = nc.dram_tensor("bq32", [D], F32, kind="ExternalInput").ap()
    bk_d = nc.dram_tensor("bk32", [D], F32, kind="ExternalInput").ap()
    b1_d = nc.dram_tensor("b1f", [F], F32, kind="ExternalInput").ap()
    br_d = nc.dram_tensor("brows", [3 * D], BF16, kind="ExternalInput").ap()

    with tile.TileContext(nc) as tc:
        with tc.tile_pool(name="const", bufs=1) as cpool, \
             tc.tile_pool(name="resid", bufs=1) as rpool, \
             tc.tile_pool(name="fm", bufs=1) as fmpool, \
             tc.tile_pool(name="qkv", bufs=1) as qkpool, \
             tc.tile_pool(name="attw", bufs=1) as apool, \
             tc.tile_pool(name="mlp", bufs=1) as mpool, \
             tc.tile_pool(name="wstr", bufs=1) as wpool, \
             tc.tile_pool(name="lnp", bufs=1) as lnpool, \
             tc.tile_pool(name="ostg", bufs=1) as opool, \
             tc.tile_pool(name="psA", bufs=4, space="PSUM") as psA:

            # ---- constants / small params ----
            cA = cpool.tile([P, 2 * KK + FK], F32, tag="cA")
            bq_sb = cA[:, 0:KK]
            bk_sb = cA[:, KK:2 * KK]
            b1_sb = cA[:, 2 * KK:2 * KK + FK]
            nc.sync.dma_start(bq_sb, bq_d.rearrange("(m p) -> p m", p=P))
            nc.sync.dma_start(bk_sb, bk_d.rearrange("(m p) -> p m", p=P))
            nc.sync.dma_start(b1_sb, b1_d.rearrange("(m p) -> p m", p=P))

            cB = cpool.tile([P, P + 2], F32, tag="cB")
            ident_f = cB[:, 0:P]
            epsap = cB[:, P:P + 1]
            expb = cB[:, P + 1:P + 2]
            make_identity(nc, ident_f)
            nc.vector.memset(epsap, EPS)
            nc.vector.memset(expb, EXPB)

            ident_b = cpool.tile([P, P], BF16, tag="identb")
            nc.vector.tensor_copy(ident_b[:], ident_f)
            ones_b = cpool.tile([1, P], BF16, tag="onesb")
            nc.vector.memset(ones_b[:], 1.0)

            cD = cpool.tile([1, 3 * D], BF16, tag="cD")
            nc.sync.dma_start(cD[:], br_d[None, :])
            t_bv = cD[:, 0:D]          # 32*bv'
            t_bo = cD[:, D:2 * D]      # 512*bo
            t_b2 = cD[:, 2 * D:3 * D]  # b2

            st = [dict() for _ in range(BL)]

            # =============== LN helpers (g/b folded into weights) ==========
            def ln_stats_new():
                stats = lnpool.tile([P, 20], F32, tag="stats", bufs=4)
                nc.vector.memset(stats[:, 0:5], 0.0)
                nc.vector.memset(stats[:, 5:10], 1.0)
                return stats

            def ln_tile_stats(stats, src, ti, pt):
                negmu = stats[:, 0:5]
                nc.vector.tensor_reduce(
                    negmu[:pt, ti:ti + 1], src[:pt, ti],
                    mybir.AxisListType.X, OP.add)
                nc.vector.tensor_scalar_mul(
                    negmu[:pt, ti:ti + 1], negmu[:pt, ti:ti + 1], -1.0 / D)
                scr = lnpool.tile([P, D], BF16, tag="xnt", bufs=3)
                nc.scalar.activation(
                    scr[:pt], src[:pt, ti], AF.Square,
                    bias=negmu[:pt, ti:ti + 1],
                    accum_out=stats[:pt, 5 + ti:5 + ti + 1])

            def ln_finalize(stats, lo, hi):
                # rsig = exp(-0.5*ln(varD/D + eps)); Ln+Exp live in the same
                # Act LUT set as softmax's Exp -> no table reload
                nc.scalar.activation(stats[:, 10 + lo:10 + hi],
                                     stats[:, 5 + lo:5 + hi], AF.Ln,
                                     scale=1.0 / D, bias=epsap[:])
                nc.scalar.activation(stats[:, 15 + lo:15 + hi],
                                     stats[:, 10 + lo:10 + hi], AF.Exp,
                                     scale=-0.5)

            def ln_apply_tile(stats, src, ti, dst_hi, dst_lo=None):
                t0, pt = TT[ti]
                negmu = stats[:, 0:5]
                rsig = stats[:, 15:20]
                xn = lnpool.tile([P, D], BF16, tag="xnt", bufs=3)
                nc.vector.tensor_scalar(
                    xn[:pt], src[:pt, ti],
                    negmu[:pt, ti:ti + 1], rsig[:pt, ti:ti + 1],
                    OP.add, OP.mult)
                for kk in range(KK):
                    pst = psA.tile([P, 1024], BF16, tag="pA")
                    nc.tensor.transpose(
                        pst[:, :pt], xn[:pt, kk * P:(kk + 1) * P],
                        ident_b[:pt, :pt])
                    nc.vector.tensor_copy(dst_hi[:, kk, t0:t0 + pt],
                                          pst[:, :pt])
                    if dst_lo is not None:
                        nc.vector.tensor_tensor(
                            dst_lo[:, kk, t0:t0 + pt], pst[:, :pt],
                            dst_hi[:, kk, t0:t0 + pt], OP.subtract)

            # =============== per-batch stage emitters ======================
            def units_load_x(b):
                us = []

                def alloc():
                    xb = rpool.tile([P, 5, D], F32, tag="xb", bufs=2)
                    st[b]["xb"] = xb
                    st[b]["stats1"] = ln_stats_new()
                    nc.vector.memset(xb[64:, 4, :], 0.0)
                us.append(alloc)
                for ti, (t0, pt) in enumerate(TT):
                    def u(ti=ti, t0=t0, pt=pt):
                        rp = min(pt, S - t0)
                        nc.sync.dma_start(st[b]["xb"][:rp, ti],
                                          x_d[b, t0:t0 + rp, :])
                        ln_tile_stats(st[b]["stats1"], st[b]["xb"], ti, pt)
                    us.append(u)
                return us

            def units_ln1_apply(b):
                # also converts the residual to bf16 (xr) on gpsimd
                us = []

                def alloc():
                    st[b]["xn"] = fmpool.tile([P, KK, SPAD], FP8,
                                              tag="xnl", bufs=4, name="xn")
                    ln_finalize(st[b]["stats1"], 0, 5)
                us.append(alloc)
                for ti, (t0, pt) in enumerate(TT):
                    def u(ti=ti, t0=t0, pt=pt):
                        ln_apply_tile(st[b]["stats1"], st[b]["xb"], ti,
                                      st[b]["xn"])
                    us.append(u)
                return us

            def units_qkv(b):
                us = []

                def alloc():
                    st[b]["q"] = qkpool.tile([P, KK, SP], FP8, tag="q", bufs=2, name="qf")
                    st[b]["k"] = qkpool.tile([P, KK, SP], FP8, tag="k", bufs=2, name="kf")
                    v = qkpool.tile([P, 5, H * VS], FP8, tag="v", bufs=2)
                    st[b]["v"] = v
                    vh = v[:].rearrange("p t (h c) -> p t h c", c=VS)
                    nc.vector.memset(vh[64:, 4:5], 0.0)
                    nc.vector.memset(vh[:, :, :, 65:66], 0.0)
                    nc.vector.memset(vh[:, 0:4, :, 64:65], 1.0 / CTXS)
                    nc.vector.memset(vh[:65, 4:5, :, 64:65], 1.0 / CTXS)
                us.append(alloc)

                def qk_units(w_d, bias_sb, dstname):
                    uu = []
                    for blk in range(2):
                        def dma(blk=blk, w_d=w_d):
                            wb = wpool.tile([P, KK, 512], FP8, tag="wblk",
                                            bufs=2)
                            st[b]["_wb"] = wb
                            nc.sync.dma_start(
                                wb[:], w_d[:, :, blk * 512:(blk + 1) * 512])
                        uu.append(dma)
                        for mi in range(4):
                            for (q0, qn) in QC:
                                def u(blk=blk, mi=mi, q0=q0, qn=qn,
                                      bias_sb=bias_sb, dstname=dstname):
                                    m = blk * 2 + mi
                                    wb = st[b]["_wb"]
                                    ps = psA.tile([P, 512], F32, tag="pA")
                                    for j in range(4):
                                        nc.tensor.matmul(
                                            ps[:, :qn],
                                            wb[:, 2 * j:2 * j + 2,
                                               mi * P:(mi + 1) * P],
                                            st[b]["xn"][:, 2 * j:2 * j + 2,
                                                        q0:q0 + qn],
                                            start=(j == 0), stop=(j == 3),
                                            perf_mode=DR)
                                    nc.vector.tensor_scalar(
                                        st[b][dstname][:, m, q0:q0 + qn],
                                        ps[:, :qn], bias_sb[:, m:m + 1],
                                        1.0 / WS, OP.add, OP.mult)
                                uu.append(u)
                    return uu

                us += qk_units(wq_d, bq_sb, "q")
                us += qk_units(wk_d, bk_sb, "k")
                # V: token-major out; xn stationary, wv moving
                for ci, (c0, cn) in enumerate(DC4):
                    def dma(c0=c0, cn=cn):
                        wb = wpool.tile([P, KK, 256], FP8, tag="wblk", bufs=2)
                        st[b]["_wb"] = wb
                        nc.sync.dma_start(wb[:], wv_d[:, :, c0:c0 + cn])
                    us.append(dma)
                    for ti, (t0, pt) in enumerate(TT):
                        def u(ci=ci, c0=c0, cn=cn, ti=ti, t0=t0, pt=pt):
                            wb = st[b]["_wb"]
                            ps = psA.tile([P, 512], F32, tag="pA")
                            nc.tensor.matmul(
                                ps[:pt, :cn], ones_b[:, :pt],
                                t_bv[:, c0:c0 + cn], start=True, stop=False)
                            for j in range(4):
                                nc.tensor.matmul(
                                    ps[:pt, :cn],
                                    st[b]["xn"][:, 2 * j:2 * j + 2, t0:t0 + pt],
                                    wb[:, 2 * j:2 * j + 2, :cn],
                                    start=False, stop=(j == 3), perf_mode=DR)
                            rp = min(pt, S - t0)
                            vh = st[b]["v"][:rp, ti].rearrange(
                                "p (h c) -> p h c", c=VS)
                            nc.vector.tensor_scalar_mul(
                                vh[:, ci * 4:(ci + 1) * 4, 0:64],
                                ps[:rp, :cn].rearrange("p (h c) -> p h c",
                                                       c=64),
                                1.0 / WS)
                        us.append(u)
                return us

            def units_attn(b):
                us = []

                def alloc():
                    st[b]["ctx"] = fmpool.tile([P, KK, SPAD], FP8,
                                               tag="ctx", bufs=2, name="ctx")
                us.append(alloc)
                for h in range(H):
                    for qi, (q0, qn) in enumerate(QC):
                        def u(h=h, q0=q0, qn=qn):
                            hrow = (h % 2) * 64
                            kkh = h // 2
                            q_fm, k_fm = st[b]["q"], st[b]["k"]
                            es = apool.tile([P, 5, ESP], FP8, tag="es",
                                            bufs=2)
                            for pair in ((0, 1), (2, 3), (4,)):
                                pg = psA.tile([P, 2, 512], F32, tag="pS",
                                              bufs=2)
                                for j, kt in enumerate(pair):
                                    t0, ptk = TT[kt]
                                    nc.tensor.matmul(
                                        pg[:ptk, j, :qn],
                                        k_fm[hrow:hrow + 64, kkh,
                                             t0:t0 + ptk],
                                        q_fm[hrow:hrow + 64, kkh,
                                             q0:q0 + qn],
                                        start=True, stop=True)
                                npair = len(pair)
                                prow = TT[pair[0]][1]
                                nc.scalar.activation(
                                    es[:prow, pair[0]:pair[0] + npair, :qn],
                                    pg[:prow, :npair, :qn],
                                    AF.Exp, scale=1.0 / np.sqrt(DH),
                                    bias=expb[:prow])
                            pc = psA.tile([P, 512], F32, tag="pA")
                            vv = st[b]["v"]
                            for pi, pair in enumerate(((0, 1), (2, 3))):
                                t0, ptk = TT[pair[0]]
                                nc.tensor.matmul(
                                    pc[:VS, :qn],
                                    vv[:ptk, pair[0]:pair[0] + 2,
                                       h * VS:(h + 1) * VS],
                                    es[:ptk, pair[0]:pair[0] + 2, :qn],
                                    start=(pi == 0), stop=False,
                                    perf_mode=DR)
                            nc.tensor.matmul(
                                pc[:VS, :qn],
                                vv[:66, 4, h * VS:(h + 1) * VS],
                                es[:66, 4, :qn],
                                start=False, stop=True)
                            rc = apool.tile([1, ESP], F32, tag="rc", bufs=2)
                            nc.vector.reciprocal(rc[:, :qn], pc[64:65, :qn])
                            rb = apool.tile([64, ESP], F32, tag="rb", bufs=2)
                            nc.gpsimd.partition_broadcast(rb[:, :qn],
                                                          rc[:, :qn])
                            nc.vector.tensor_tensor(
                                st[b]["ctx"][hrow:hrow + 64, kkh,
                                             q0:q0 + qn],
                                pc[0:64, :qn], rb[:, :qn], OP.mult)
                        us.append(u)
                return us

            def units_o(b):
                us = []

                def alloc():
                    st[b]["x2"] = rpool.tile([P, 5, D], BF16, tag="x2",
                                             bufs=2, name="x2")
                    st[b]["stats2"] = ln_stats_new()
                us.append(alloc)
                for ci, (c0, cn) in enumerate(DC4):
                    def dma(c0=c0, cn=cn):
                        wb = wpool.tile([P, KK, 256], FP8, tag="wblk", bufs=2)
                        st[b]["_wb"] = wb
                        nc.sync.dma_start(wb[:], wo_d[:, :, c0:c0 + cn])
                    us.append(dma)
                    for ti, (t0, pt) in enumerate(TT):
                        def u(ci=ci, c0=c0, cn=cn, ti=ti, t0=t0, pt=pt):
                            wb = st[b]["_wb"]
                            ps = psA.tile([P, 512], F32, tag="pA")
                            nc.tensor.matmul(
                                ps[:pt, :cn], ones_b[:, :pt],
                                t_bo[:, c0:c0 + cn], start=True, stop=False)
                            for j in range(4):
                                nc.tensor.matmul(
                                    ps[:pt, :cn],
                                    st[b]["ctx"][:, 2 * j:2 * j + 2,
                                                 t0:t0 + pt],
                                    wb[:, 2 * j:2 * j + 2, :cn],
                                    start=False, stop=(j == 3), perf_mode=DR)
                            nc.vector.scalar_tensor_tensor(
                                st[b]["x2"][:pt, ti, c0:c0 + cn],
                                ps[:pt, :cn], 1.0 / (WS * CTXS),
                                st[b]["xb"][:pt, ti, c0:c0 + cn],
                                OP.mult, OP.add)
                            if ci == len(DC4) - 1:
                                ln_tile_stats(st[b]["stats2"], st[b]["x2"],
                                              ti, pt)
                        us.append(u)
                return us

            def units_ln2_apply(b):
                us = []

                def alloc():
                    st[b]["xn2h"] = fmpool.tile([P, KK, SPAD], FP8,
                                                tag="xnl", bufs=4, name="xn2h")
                    st[b]["xn2l"] = fmpool.tile([P, KK, SPAD], FP8,
                                                tag="xnl", bufs=4, name="xn2l")
                    ln_finalize(st[b]["stats2"], 0, 5)
                us.append(alloc)
                for ti in range(5):
                    def u(ti=ti):
                        ln_apply_tile(st[b]["stats2"], st[b]["x2"], ti,
                                      st[b]["xn2h"], st[b]["xn2l"])
                    us.append(u)
                return us

            def units_w1(b):
                us = []

                def alloc():
                    st[b]["h1"] = mpool.tile([P, FK, SP], BF16, tag="h1",
                                             bufs=1, name="h1")
                us.append(alloc)
                for blk in range(8):
                    def dma(blk=blk):
                        wh = wpool.tile([P, KK, 512], FP8, tag="w1h", bufs=2)
                        wl = wpool.tile([P, KK, 512], FP8, tag="w1l", bufs=2)
                        st[b]["_w1h"], st[b]["_w1l"] = wh, wl
                        nc.sync.dma_start(
                            wh[:], w1h_d[:, :, blk * 512:(blk + 1) * 512])
                        nc.sync.dma_start(
                            wl[:], w1l_d[:, :, blk * 512:(blk + 1) * 512])
                    us.append(dma)
                    for mi in range(4):
                        for (q0, qn) in QC:
                            def u(blk=blk, mi=mi, q0=q0, qn=qn):
                                m = blk * 4 + mi
                                wh, wl = st[b]["_w1h"], st[b]["_w1l"]
                                xh, xl = st[b]["xn2h"], st[b]["xn2l"]
                                ps = psA.tile([P, 512], F32, tag="pA")
                                first = True
                                for j in range(4):
                                    wsl = (slice(None),
                                           slice(2 * j, 2 * j + 2),
                                           slice(mi * P, (mi + 1) * P))
                                    xsl = (slice(None),
                                           slice(2 * j, 2 * j + 2),
                                           slice(q0, q0 + qn))
                                    for wt, xt in ((wh, xh), (wl, xh),
                                                   (wh, xl)):
                                        nc.tensor.matmul(
                                            ps[:, :qn], wt[wsl], xt[xsl],
                                            start=first,
                                            stop=(j == 3 and xt is xl),
                                            perf_mode=DR)
                                        first = False
                                nc.scalar.activation(
                                    st[b]["h1"][:, m, q0:q0 + qn],
                                    ps[:, :qn], _GELU,
                                    bias=b1_sb[:, m:m + 1], scale=1.0 / WS)
                            us.append(u)
                return us

            def units_w2(b):
                us = []
                for ci in range(D // PW2):
                    def dma(ci=ci):
                        wb = wpool.tile([P, FK, PW2], BF16, tag="w2", bufs=2)
                        st[b]["_w2"] = wb
                        nc.sync.dma_start(
                            wb[:], w2_d[:, :, ci * PW2:(ci + 1) * PW2])
                    us.append(dma)
                    for ti, (t0, pt) in enumerate(TT):
                        def u(ci=ci, ti=ti, t0=t0, pt=pt):
                            wb = st[b]["_w2"]
                            ps = psA.tile([P, 512], F32, tag="pA")
                            nc.tensor.matmul(
                                ps[:pt, :PW2], ones_b[:, :pt],
                                t_b2[:, ci * PW2:(ci + 1) * PW2],
                                start=True, stop=False)
                            for fk in range(FK):
                                nc.tensor.matmul(
                                    ps[:pt, :PW2],
                                    st[b]["h1"][:, fk, t0:t0 + pt],
                                    wb[:, fk, :],
                                    start=False, stop=(fk == FK - 1))
                            rp = min(pt, S - t0)
                            og = opool.tile([P, PW2], F32, tag="og", bufs=4)
                            nc.vector.scalar_tensor_tensor(
                                og[:pt], ps[:pt, :PW2], 1.0,
                                st[b]["x2"][:pt, ti, ci * PW2:(ci + 1) * PW2],
                                OP.mult, OP.add)
                            nc.sync.dma_start(
                                y_d[b, t0:t0 + rp, ci * PW2:(ci + 1) * PW2],
                                og[:rp])
                        us.append(u)
                return us

            # =============== emission schedule =============================
            def emit(units):
                for u in units:
                    u()

            def interleave(primary, fillers, ratio):
                """Emit primary units; after each, emit `ratio` filler units."""
                fi = 0
                acc = 0.0
                for u in primary:
                    u()
                    acc += ratio
                    while fi < len(fillers) and acc >= 1.0:
                        fillers[fi]()
                        fi += 1
                        acc -= 1.0
                while fi < len(fillers):
                    fillers[fi]()
                    fi += 1

            # batch 0 head of pipeline
            emit(units_load_x(0))
            emit(units_ln1_apply(0))
            emit(units_qkv(0))

            # attention(0) with batch-1 load/LN1/QKV as PE filler
            fill1 = (units_load_x(1) + units_ln1_apply(1) + units_qkv(1))
            interleave(units_attn(0), fill1, ratio=len(fill1) / 32.0)

            emit(units_o(0))
            emit(units_ln2_apply(0))

            # MLP(0) with attention(1) spread through it; keep gelu/exp
            # table switches coarse: chunks of w1 units between attn pairs
            w1u = units_w1(0)
            w2u = units_w2(0)
            at1 = units_attn(1)
            emit([at1[0]])          # ctx alloc
            at1 = at1[1:]
            # w1 phase: 3 bursts of attention between w1 chunks
            emit(w1u[:23])
            emit(at1[0:4])
            emit(w1u[23:46])
            emit(at1[4:8])
            emit(w1u[46:])
            emit(at1[8:12])
            # w2 phase: no Act work in w2 -> free interleave
            interleave(w2u, at1[12:], ratio=len(at1[12:]) / len(w2u))

            emit(units_o(1))
            emit(units_ln2_apply(1))
            emit(units_w1(1))
            emit(units_w2(1))

    nc.compile()
    return nc


def _get_nc():
    global _NC_CACHE
    if _NC_CACHE is None:
        _NC_CACHE = _build()
    return _NC_CACHE


def _q8(a):
    return np.ascontiguousarray(a.astype(np.float32)).astype(
        ml_dtypes.float8_e4m3)


def _rearr(a, k):
    # [(k p), n] -> [p, k, n]
    n = a.shape[-1]
    return np.ascontiguousarray(
        a.reshape(k, P, n).transpose(1, 0, 2))


def prep_shared(inputs):
    """Host-side weight prep: LN folding, fp8 scaling/splitting, layouts."""
    i = {k: np.asarray(v, np.float64) for k, v in inputs.items()}
    g1, gb1 = i["ln1_g"], i["ln1_b"]
    g2, gb2 = i["ln2_g"], i["ln2_b"]

    out = {}
    for name, wname, bname in (("q", "wq", "bq"), ("k", "wk", "bk"),
                               ("v", "wv", "bv")):
        wf = g1[:, None] * i[wname]
        bf = i[bname] + gb1 @ i[wname]
        out["w" + name + "8"] = _rearr(_q8(WS * wf), KK)
        if name == "v":
            bv32 = (WS * bf).astype(ml_dtypes.bfloat16)
        else:
            out["b" + name + "32"] = (WS * bf).astype(np.float32)
    out["wo8"] = _rearr(_q8(WS * i["wo"]), KK)
    bo512 = (WS * CTXS * i["bo"]).astype(ml_dtypes.bfloat16)

    w1f = WS * (g2[:, None] * i["w1"])
    w1h = _q8(w1f)
    w1l = _q8(w1f - w1h.astype(np.float64))
    out["w1h"] = _rearr(w1h, KK)
    out["w1l"] = _rearr(w1l, KK)
    out["b1f"] = (i["b1"] + gb2 @ i["w1"]).astype(np.float32)
    out["w2b"] = _rearr(
        np.ascontiguousarray(i["w2"].astype(np.float32)).astype(
            ml_dtypes.bfloat16), FK)
    b2 = i["b2"].astype(ml_dtypes.bfloat16)
    out["brows"] = np.ascontiguousarray(
        np.concatenate([bv32, bo512, b2]))
    return out


def kernel(**inputs):
    nc = _get_nc()
    shared = prep_shared(inputs)
    x = np.ascontiguousarray(np.asarray(inputs["x"], dtype=np.float32))
    in_maps = []
    for i in range(NCORES):
        m = dict(shared)
        m["x"] = np.ascontiguousarray(x[i * BL:(i + 1) * BL])
        in_maps.append(m)
    res = bass_utils.run_bass_kernel_spmd(nc, in_maps,
                                          core_ids=list(range(NCORES)))
    y = np.concatenate([res.results[i]["y"] for i in range(NCORES)], axis=0)
    return y.astype(np.float32)
